# revision 29
# baseline (speedup 1.0000x reference)
"""Trainium2 Bass kernel for nn_BottomUpIntegrator (gnn_message_passing).

Sharding: cells split at cluster boundaries across 8 cores (2048 clusters each);
per-core segment sums via one-hot scatter matmuls into PSUM with a core-invariant
static window schedule; cluster+organism phase on-chip; host combines 12 organism
partial floats per core into the final 6 self-model outputs.

v4: single ACT table set (sigmoid via tanh), relu split ACT/DVE, one-hot shipped
from host as fp8 (exact 0/1; PE takes bf16 weights x fp8 moving), static scatter
values precomputed on host, scatter software-pipelined one chunk behind, single
PSUM bank for all 4 scatter accumulators (32-aligned partition groups), and the
cluster phase (phase B) executed per 512-cluster block pipelined INSIDE phase A:
all its transposes run on DVE StreamTranspose (32x32 blocks), the cluster-MLP
second layer lands cluster-major via activations-as-stationary matmuls, so only
the organism reductions remain in the tail.
"""
import numpy as np
import ml_dtypes

import json as _json

from concourse import bass, mybir
from concourse import bass2jax as _b2j
from concourse import bass_utils as _bu
from concourse.tile import TileContext
from concourse.bass_utils import run_bass_kernel_spmd

_orig_compile = _bu.compile_bir_kernel


def _split_waits_compile(bir_json, tmpdir, neff_name="file.neff"):
    """Walrus lowers at most ONE semaphore wait per TPB instruction struct.
    Tile emits several. Hoist extras onto injected same-engine EventSemaphore
    wait instructions immediately before the owner (semantically identical:
    engines execute in program order)."""
    d = _json.loads(bir_json)
    cnt = 0
    for fn in d["functions"]:
        for blk in fn["blocks"]:
            newlist = []
            for ins in blk["instructions"]:
                si = ins.get("sync_info")
                waits = si.get("on_wait", []) if si else []
                if si and len(waits) > 1 and ins.get("opcode") not in (
                        "EventSemaphore",):
                    for w_i, w in enumerate(waits[:-1]):
                        cnt += 1
                        newlist.append({
                            "debug": ins.get("debug", 0),
                            "engine": ins["engine"],
                            "ins": [], "outs": [],
                            "name": f"{ins['name']}-wsplit{w_i}",
                            "opcode": "EventSemaphore",
                            "sync_info": {"on_update": [], "on_wait": [w]},
                        })
                    si["on_wait"] = [waits[-1]]
                newlist.append(ins)
            blk["instructions"] = newlist
    print(f"[wait-split] hoisted {cnt} extra waits")
    return _orig_compile(_json.dumps(d).encode(), tmpdir, neff_name=neff_name)


_bu.compile_bir_kernel = _split_waits_compile
_b2j.compile_bir_kernel = _split_waits_compile

F32 = mybir.dt.float32
BF16 = mybir.dt.bfloat16
FP8 = mybir.dt.float8e4
AF = mybir.ActivationFunctionType
OP = mybir.AluOpType
AX = mybir.AxisListType

NCORES = 8
KLOC = 2048            # clusters per core
NPAD = 262144          # padded cells per core
CHUNK = 8192           # cells per chunk
NCHUNK = NPAD // CHUNK # 32
W = 32                 # onehot window width (clusters)
NTILES = NPAD // 128   # 2048 scatter tiles per core
TPB = NTILES // 4      # tiles per 512-cluster block
PADSEG = 1.0e9
DVE_RELU = (2, 5, 7)   # relu tiles evacuated on DVE instead of ACT


def _window_start(S):
    s = S % TPB
    return int(np.clip(s - 16, 0, 512 - W))


def build_program():
    nc = bass.Bass(trn_type="TRN2", use_seq_codegen=True)
    featsT = nc.dram_tensor("featsT", [72, NPAD // 2], BF16, kind="ExternalInput")
    vtstat = nc.dram_tensor("vtstat", [NCHUNK, 128, 960], BF16, kind="ExternalInput")
    cellvec = nc.dram_tensor("cellvec", [NCHUNK, 128, 128], BF16, kind="ExternalInput")
    ohdram = nc.dram_tensor("ohdram", [NCHUNK, 128, 2048], FP8, kind="ExternalInput")
    w1d = nc.dram_tensor("w1d", [72, 128], BF16, kind="ExternalInput")
    b1d = nc.dram_tensor("b1d", [128, 1], F32, kind="ExternalInput")
    w2d = nc.dram_tensor("w2d", [128, 2], BF16, kind="ExternalInput")
    b2n = nc.dram_tensor("b2n", [128, 1], F32, kind="ExternalInput")
    v1 = nc.dram_tensor("v1", [7, 32], F32, kind="ExternalInput")
    c1b = nc.dram_tensor("c1b", [32, 1], F32, kind="ExternalInput")
    v2 = nc.dram_tensor("v2", [32, 1], F32, kind="ExternalInput")
    c2n = nc.dram_tensor("c2n", [128, 1], F32, kind="ExternalInput")
    out_cluster = nc.dram_tensor("out_cluster", [KLOC, 8], F32, kind="ExternalOutput")
    out_org = nc.dram_tensor("out_org", [1, 12], F32, kind="ExternalOutput")

    with TileContext(nc) as tc:
        with (
            tc.tile_pool(name="const", bufs=1) as cp,
            tc.tile_pool(name="feats", bufs=3) as fp,
            tc.tile_pool(name="hs", bufs=2) as hp,
            tc.tile_pool(name="small", bufs=4) as sp,
            tc.tile_pool(name="scatv", bufs=3) as vp,
            tc.tile_pool(name="ph_b", bufs=1) as bp,
            tc.tile_pool(name="scatps", bufs=1, space="PSUM") as pps,
            tc.tile_pool(name="mm1ps", bufs=4, space="PSUM") as pp1,
            tc.tile_pool(name="mm2ps", bufs=1, space="PSUM") as pp2,
            tc.tile_pool(name="mmbps", bufs=1, space="PSUM") as ppm,
        ):
            # ---- constants ----------------------------------------------
            w1s = cp.tile([72, 128], BF16, tag="w1s")
            nc.sync.dma_start(out=w1s[:], in_=w1d[:])
            b1s = cp.tile([128, 1], F32, tag="b1s")
            nc.sync.dma_start(out=b1s[:], in_=b1d[:])
            w2s = cp.tile([128, 2], BF16, tag="w2s")
            nc.sync.dma_start(out=w2s[:], in_=w2d[:])
            b2ns = cp.tile([128, 1], F32, tag="b2ns")
            nc.sync.dma_start(out=b2ns[:], in_=b2n[:])
            v1s = cp.tile([7, 32], F32, tag="v1s")
            nc.sync.dma_start(out=v1s[:], in_=v1[:])
            c1s = cp.tile([32, 1], F32, tag="c1s")
            nc.sync.dma_start(out=c1s[:], in_=c1b[:])
            v2s = cp.tile([32, 1], F32, tag="v2s")
            nc.sync.dma_start(out=v2s[:], in_=v2[:])
            c2ns = cp.tile([128, 1], F32, tag="c2ns")
            nc.sync.dma_start(out=c2ns[:], in_=c2n[:])
            ones = cp.tile([128, 1], F32, tag="ones")
            nc.vector.memset(ones[:], 1.0)

            zbf = cp.tile([128, 512], BF16, tag="zbf")
            nc.vector.memset(zbf[:], 0.0)

            # Pre-touch DMA-loaded constants on their consuming engines so no
            # later compute instruction needs a second (DMA) semaphore wait.
            scra = cp.tile([128, 4], F32, tag="scra")
            nc.scalar.activation(out=scra[:, 0:1], in_=b1s[:], func=AF.Copy)
            nc.scalar.activation(out=scra[:, 1:2], in_=b2ns[:], func=AF.Copy)
            nc.scalar.activation(out=scra[0:32, 2:3], in_=c1s[:], func=AF.Copy)
            nc.scalar.activation(out=scra[:, 3:4], in_=c2ns[:], func=AF.Copy)
            scrv = cp.tile([1, 1], F32, tag="scrv")
            nc.vector.tensor_copy(out=scrv[:], in_=b1s[0:1, 0:1])

            # persistent scatter accumulators: ONE PSUM bank; block b's
            # [15, 512] accumulator lives at partitions 32b..32b+15.
            scatb = pps.tile([128, 512], F32, tag="scatb", name="scatb")
            scat = [scatb[32 * b:32 * b + 15, :] for b in range(4)]
            # PE touch of w1s (rides on Ldweights; result overwritten below).
            nc.tensor.matmul(out=scatb[0:1, 0:1], lhsT=w1s[0:1, 0:1],
                             rhs=zbf[0:1, 0:1], start=True, stop=True,
                             skip_group_check=True)
            nc.tensor.matmul(out=scatb[:], lhsT=zbf[:, 0:128], rhs=zbf[:],
                             start=True, stop=False, skip_group_check=True)

            # ---- persistent phase-B tiles -------------------------------
            scP = cp.tile([32, 2048], F32, tag="scP")      # PSUM evac (15 rows used)
            tt32 = cp.tile([128, 512], F32, tag="tt32")    # [p, ct*32+v]
            cftt32 = cp.tile([32, 2048], F32, tag="cftt32")  # [q(7 used), cluster]
            aggP = cp.tile([128, 64], F32, tag="aggP")
            gaP = cp.tile([128, 64], F32, tag="gaP")
            bvP = cp.tile([128, 64], F32, tag="bvP")
            phicP = cp.tile([128, 16], F32, tag="phicP")
            cohP = cp.tile([128, 16], F32, tag="cohP")
            perrP = cp.tile([128, 16], F32, tag="perrP")
            integP = cp.tile([128, 16], F32, tag="integP")
            validP = cp.tile([128, 16], F32, tag="validP")
            eimpP = cp.tile([128, 16], F32, tag="eimpP")
            basecP = cp.tile([128, 16], F32, tag="basecP")
            pvP = cp.tile([128, 16], F32, tag="pvP")
            cvvP = cp.tile([128, 16], F32, tag="cvvP")

            pbstate = {}

            def lt(tag, w=4):
                return bp.tile([128, w], F32, tag=tag, name=tag)

            def pb_part0(b):
                # PSUM evac + tt transposes (DVE only)
                nc.vector.tensor_copy(out=scP[0:15, 512 * b:512 * (b + 1)],
                                      in_=scat[b])
                for t in range(4):
                    ct = 4 * b + t
                    for i in range(4):
                        o = 512 * b + 128 * t + 32 * i
                        nc.vector.transpose(
                            out=tt32[32 * i:32 * i + 32, 32 * ct:32 * ct + 32],
                            in_=scP[0:32, o:o + 32])

            def pb_part1(b):
                # per-cluster stats for this block's 4 cluster-tiles
                B4 = slice(4 * b, 4 * (b + 1))
                B16 = slice(16 * b, 16 * (b + 1))
                tv = tt32[:, 128 * b:128 * (b + 1)].rearrange(
                    "p (t v) -> p t v", v=32)
                cnt = tv[:, :, 0:1]
                sew = tv[:, :, 1:2]
                sewa = tv[:, :, 2:6]
                sa = tv[:, :, 6:10]
                ssq = tv[:, :, 10:14]
                ssur = tv[:, :, 14:15]

                def v3(t):
                    return t[:].rearrange("p (b a) -> p b a", a=1)

                def v44(t):
                    return t[:].rearrange("p (b a) -> p b a", a=4)

                cntc = lt("cntc")
                nc.vector.tensor_scalar(out=v3(cntc), in0=cnt, scalar1=1.0,
                                        scalar2=None, op0=OP.max)
                rc = lt("rc")
                nc.vector.reciprocal(out=rc[:], in_=cntc[:])
                den = lt("den")
                nc.vector.tensor_scalar(out=v3(den), in0=sew, scalar1=1.0,
                                        scalar2=None, op0=OP.max)
                rden = lt("rden")
                nc.vector.reciprocal(out=rden[:], in_=den[:])
                agr = lt("agr", 16)
                nc.vector.tensor_tensor(out=v44(agr), in0=sewa,
                                        in1=rden[:].to_broadcast([128, 4, 4]),
                                        op=OP.mult)
                mx = lt("mx")
                nc.vector.tensor_reduce(out=v3(mx), in_=v44(agr), axis=AX.X,
                                        op=OP.max)
                es = lt("es", 16)
                nc.vector.tensor_tensor(out=v44(es), in0=v44(agr),
                                        in1=mx[:].to_broadcast([128, 4, 4]),
                                        op=OP.subtract)
                nc.scalar.activation(out=es[:], in_=es[:], func=AF.Exp)
                ssum = lt("ssum")
                nc.vector.tensor_reduce(out=v3(ssum), in_=v44(es), axis=AX.X,
                                        op=OP.add)
                rssum = lt("rssum")
                nc.vector.reciprocal(out=rssum[:], in_=ssum[:])
                aggv = aggP[:, B16].rearrange("p (b a) -> p b a", a=4)
                nc.vector.tensor_tensor(out=aggv, in0=v44(es),
                                        in1=rssum[:].to_broadcast([128, 4, 4]),
                                        op=OP.mult)
                mean = lt("mean", 16)
                nc.vector.tensor_tensor(out=v44(mean), in0=sa,
                                        in1=rc[:].to_broadcast([128, 4, 4]),
                                        op=OP.mult)
                var = lt("var", 16)
                nc.vector.tensor_tensor(out=v44(var), in0=v44(mean),
                                        in1=v44(mean), op=OP.mult)
                cntb = lt("cntb")
                nc.vector.tensor_copy(out=v3(cntb), in_=cnt)
                nc.vector.tensor_tensor(out=v44(var), in0=v44(var),
                                        in1=cntb[:].to_broadcast([128, 4, 4]),
                                        op=OP.mult)
                nc.vector.tensor_tensor(out=v44(var), in0=ssq, in1=v44(var),
                                        op=OP.subtract)
                cm1 = lt("cm1")
                nc.vector.tensor_scalar(out=v3(cm1), in0=cnt, scalar1=-1.0,
                                        scalar2=1.0, op0=OP.add, op1=OP.max)
                rcm1 = lt("rcm1")
                nc.vector.reciprocal(out=rcm1[:], in_=cm1[:])
                nc.vector.tensor_tensor(out=v44(var), in0=v44(var),
                                        in1=rcm1[:].to_broadcast([128, 4, 4]),
                                        op=OP.mult)
                vm = lt("vm")
                nc.vector.tensor_reduce(out=v3(vm), in_=v44(var), axis=AX.X,
                                        op=OP.add)
                nc.vector.tensor_scalar(out=vm[:], in0=vm[:], scalar1=0.25,
                                        scalar2=None, op0=OP.mult)
                phs = phicP[:, B4]
                nc.vector.tensor_scalar(out=phs, in0=vm[:], scalar1=2.0,
                                        scalar2=1.0, op0=OP.mult, op1=OP.min)
                nc.vector.tensor_scalar(out=phs, in0=phs, scalar1=-1.0,
                                        scalar2=1.0, op0=OP.mult, op1=OP.add)
                nc.vector.tensor_scalar(out=cohP[:, B4], in0=vm[:], scalar1=-1.0,
                                        scalar2=1.0, op0=OP.mult, op1=OP.add)
                nc.vector.tensor_tensor(
                    out=perrP[:, B4].rearrange("p (b a) -> p b a", a=1),
                    in0=ssur, in1=v3(rc), op=OP.mult)
                igs = integP[:, B4]
                nc.vector.tensor_scalar(out=igs, in0=perrP[:, B4], scalar1=-1.0,
                                        scalar2=1.0, op0=OP.mult, op1=OP.add)
                nc.vector.tensor_tensor(out=igs, in0=igs, in1=phs, op=OP.mult)
                nc.vector.tensor_scalar(
                    out=validP[:, B4].rearrange("p (b a) -> p b a", a=1),
                    in0=cnt, scalar1=0.0, scalar2=None, op0=OP.is_gt)
                szf = lt("szf")
                nc.vector.tensor_scalar(out=v3(szf), in0=cnt, scalar1=0.05,
                                        scalar2=1.0, op0=OP.mult, op1=OP.min)
                # cluster features, 32-padded per cluster-tile, then transpose
                cft32 = lt("cft32", 128)
                cfv = cft32[:].rearrange("p (t q) -> p t q", q=32)
                nc.vector.tensor_copy(out=cfv[:, :, 0:4], in_=aggv)
                nc.vector.tensor_copy(out=cfv[:, :, 4:5],
                                      in_=phs.to_broadcast([128, 4, 1]))
                nc.vector.tensor_copy(out=cfv[:, :, 5:6],
                                      in_=cohP[:, B4].to_broadcast([128, 4, 1]))
                nc.vector.tensor_copy(out=cfv[:, :, 6:7],
                                      in_=szf[:].to_broadcast([128, 4, 1]))
                for t in range(4):
                    for i in range(4):
                        nc.vector.transpose(
                            out=cftt32[0:32, 512 * b + 128 * t + 32 * i:
                                       512 * b + 128 * t + 32 * i + 32],
                            in_=cft32[32 * i:32 * i + 32, 32 * t:32 * t + 32])

            def pb_part2(b):
                B4 = slice(4 * b, 4 * (b + 1))
                B16 = slice(16 * b, 16 * (b + 1))
                aggv = aggP[:, B16].rearrange("p (b a) -> p b a", a=4)
                # cluster MLP: layer 1 cluster-free-major, layer 2 back to
                # cluster-partition-major via activations-as-stationary
                hcp = ppm.tile([32, 512], F32, tag="mmp")
                nc.tensor.matmul(out=hcp[:], lhsT=v1s[:],
                                 rhs=cftt32[0:7, 512 * b:512 * (b + 1)],
                                 start=True, stop=True)
                hcs = bp.tile([32, 512], F32, tag="hcs", name="hcs")
                nc.scalar.activation(out=hcs[:], in_=hcp[:], func=AF.Relu,
                                     bias=c1s[:])
                bcp4 = ppm.tile([128, 4], F32, tag="mmp2")
                for t in range(4):
                    nc.tensor.matmul(out=bcp4[:, t:t + 1],
                                     lhsT=hcs[:, 128 * t:128 * (t + 1)],
                                     rhs=v2s[:], start=True, stop=True,
                                     skip_group_check=True)
                bsl = basecP[:, B4]
                nc.scalar.activation(out=bsl, in_=bcp4[:], func=AF.Tanh,
                                     bias=c2ns[:], scale=0.5)
                nc.vector.tensor_scalar(out=bsl, in0=bsl, scalar1=1.0,
                                        scalar2=0.5, op0=OP.add, op1=OP.mult)
                impc = lt("impcB")
                nc.vector.tensor_tensor(out=impc[:], in0=bsl, in1=phicP[:, B4],
                                        op=OP.mult)
                nc.vector.tensor_scalar(out=impc[:], in0=impc[:], scalar1=0.01,
                                        scalar2=1.0, op0=OP.max, op1=OP.min)
                eimp0 = lt("eimp0")
                nc.scalar.activation(out=eimp0[:], in_=impc[:], func=AF.Exp)
                nc.vector.tensor_tensor(out=eimpP[:, B4], in0=eimp0[:],
                                        in1=validP[:, B4], op=OP.mult)
                # specialization one-hot with first-match-wins
                amx = lt("amx")
                nc.vector.tensor_reduce(
                    out=amx[:].rearrange("p (b a) -> p b a", a=1),
                    in_=aggv, axis=AX.X, op=OP.max)
                bselv = bvP[:, B16].rearrange("p (b a) -> p b a", a=4)
                nc.vector.tensor_tensor(out=bselv, in0=aggv,
                                        in1=amx[:].to_broadcast([128, 4, 4]),
                                        op=OP.is_equal)
                taken = lt("taken")
                nc.vector.memset(taken[:], 0.0)
                notk = lt("notk")
                for a in range(4):
                    nc.vector.tensor_scalar(out=notk[:], in0=taken[:],
                                            scalar1=-1.0, scalar2=1.0,
                                            op0=OP.mult, op1=OP.add)
                    nc.vector.tensor_tensor(
                        out=bselv[:, :, a:a + 1], in0=bselv[:, :, a:a + 1],
                        in1=notk[:].rearrange("p (b a) -> p b a", a=1),
                        op=OP.mult)
                    if a < 3:
                        nc.vector.tensor_tensor(
                            out=taken[:].rearrange("p (b a) -> p b a", a=1),
                            in0=taken[:].rearrange("p (b a) -> p b a", a=1),
                            in1=bselv[:, :, a:a + 1], op=OP.max)
                nc.vector.tensor_tensor(out=bselv, in0=bselv,
                                        in1=validP[:, B4].to_broadcast([128, 4, 4]),
                                        op=OP.mult)
                nc.vector.tensor_tensor(out=gaP[:, B16].rearrange(
                    "p (b a) -> p b a", a=4), in0=aggv,
                    in1=eimpP[:, B4].to_broadcast([128, 4, 4]), op=OP.mult)
                nc.vector.tensor_tensor(out=pvP[:, B4], in0=phicP[:, B4],
                                        in1=validP[:, B4], op=OP.mult)
                nc.vector.tensor_tensor(out=cvvP[:, B4], in0=cohP[:, B4],
                                        in1=validP[:, B4], op=OP.mult)
                # cluster_out writeback for this block
                oc32 = lt("oc32", 32)
                ocv = oc32[:].rearrange("p (t q) -> p t q", q=8)
                nc.vector.tensor_copy(out=ocv[:, :, 0:4], in_=aggv)
                nc.vector.tensor_copy(out=ocv[:, :, 4:5],
                                      in_=phicP[:, B4].to_broadcast([128, 4, 1]))
                nc.vector.tensor_copy(out=ocv[:, :, 5:6],
                                      in_=cohP[:, B4].to_broadcast([128, 4, 1]))
                nc.vector.tensor_copy(out=ocv[:, :, 6:7],
                                      in_=perrP[:, B4].to_broadcast([128, 4, 1]))
                nc.vector.tensor_copy(out=ocv[:, :, 7:8],
                                      in_=integP[:, B4].to_broadcast([128, 4, 1]))
                nc.sync.dma_start(
                    out=out_cluster[:].rearrange("(c p) q -> p c q", p=128)[
                        :, 4 * b:4 * (b + 1), :],
                    in_=ocv)

            PB_PARTS = (pb_part0, pb_part1, pb_part2)

            def emit_scatter(k, vts, oh):
                # scatter: col j -> sorted tile S = 64k + 32*(j%2) + j//2
                for j in range(64):
                    S = 64 * k + 32 * (j % 2) + (j // 2)
                    blk = S // TPB
                    f = _window_start(S)
                    nc.tensor.matmul(out=scat[blk][:, f:f + W],
                                     lhsT=vts[:, 15 * j:15 * j + 15],
                                     rhs=oh[:, 32 * j:32 * j + 32],
                                     start=False,
                                     stop=(k == NCHUNK - 1 and j >= 62),
                                     skip_group_check=True,
                                     tile_position=(0, 32 * blk))

            # ---- phase A ------------------------------------------------
            prev = None
            for k in range(NCHUNK):
                ft = fp.tile([72, 4096], BF16, tag="ft")
                nc.sync.dma_start(out=ft[:],
                                  in_=featsT[:, k * 4096:(k + 1) * 4096])
                vts = vp.tile([128, 960], BF16, tag="vts")
                nc.sync.dma_start(out=vts[:], in_=vtstat[k])
                cv = sp.tile([128, 128], BF16, tag="cv")
                nc.sync.dma_start(out=cv[:], in_=cellvec[k])
                oh = vp.tile([128, 2048], FP8, tag="oh")
                nc.sync.dma_start(out=oh[:], in_=ohdram[k])

                # mm1 + bias + relu -> h [128, 4096] bf16 (ACT/DVE split)
                hs = hp.tile([128, 4096], BF16, tag="hs")
                for j in range(8):
                    hp1 = pp1.tile([128, 512], F32, tag="hp1")
                    nc.tensor.matmul(out=hp1[:], lhsT=w1s[:],
                                     rhs=ft[:, 512 * j:512 * (j + 1)],
                                     start=True, stop=True)
                    dst = hs[:, 512 * j:512 * (j + 1)]
                    if j in DVE_RELU:
                        nc.vector.tensor_scalar(
                            out=dst, in0=hp1[:], scalar1=b1s[:],
                            scalar2=0.0, op0=OP.add, op1=OP.max)
                    else:
                        nc.scalar.activation(out=dst, in_=hp1[:],
                                             func=AF.Relu, bias=b1s[:])

                # mm2 -> base logits [128, 64] cell-major in PSUM
                bb = pp2.tile([128, 64], F32, tag="bb")
                for t in range(32):
                    nc.tensor.matmul(out=bb[:, 2 * t:2 * t + 2],
                                     lhsT=hs[:, 128 * t:128 * (t + 1)], rhs=w2s[:],
                                     start=(t == 0), stop=(t == 31),
                                     skip_group_check=True)

                # sigmoid via tanh: sig = (1 + tanh((bb+b2)/2)) / 2, so
                # sig*eph = (th + 1) * (eph/2) -- one STT op.
                # cv lanes: 0:64 eph, 64:128 eph/2
                th = sp.tile([128, 64], F32, tag="th")
                nc.scalar.activation(out=th[:], in_=bb[:], func=AF.Tanh,
                                     bias=b2ns[:], scale=0.5)
                imp = sp.tile([128, 64], F32, tag="imp")
                nc.vector.scalar_tensor_tensor(
                    out=imp[:], in0=th[:], scalar=1.0, in1=cv[:, 64:128],
                    op0=OP.add, op1=OP.mult)
                impc = sp.tile([128, 64], F32, tag="impc")
                nc.vector.tensor_scalar(out=impc[:], in0=imp[:], scalar1=0.01,
                                        scalar2=1.0, op0=OP.max, op1=OP.min)
                wc = sp.tile([128, 64], F32, tag="wc")
                nc.vector.tensor_tensor(out=wc[:], in0=impc[:], in1=cv[:, 0:64],
                                        op=OP.mult)
                ew = sp.tile([128, 64], BF16, tag="ew")
                nc.scalar.activation(out=ew[:], in_=wc[:], func=AF.Exp)

                # fill dynamic lanes of vt: [p, s, 15] lanes 1 (ew), 2:6 (ew*a)
                vv = vts[:].rearrange("p (s v) -> p s v", v=15)
                nc.vector.tensor_copy(out=vv[:, :, 1:2],
                                      in_=ew[:].to_broadcast([128, 64, 1]))
                nc.vector.tensor_tensor(out=vv[:, :, 2:6], in0=vv[:, :, 6:10],
                                        in1=ew[:].to_broadcast([128, 64, 4]),
                                        op=OP.mult)

                # scatter of the PREVIOUS chunk (software pipelining: its
                # vt/oh are long ready, so the PE never stalls mid-chunk)
                if prev is not None:
                    emit_scatter(*prev)
                prev = (k, vts, oh)

                # pipelined phase-B parts: block b's parts at k=8b+8..8b+10
                if k >= 8 and k % 8 in (0, 1, 2):
                    PB_PARTS[k % 8](k // 8 - 1)

            emit_scatter(*prev)
            for part in PB_PARTS:
                part(3)

            # ---- organism tail ------------------------------------------
            r = bp.tile([128, 12], F32, tag="r")
            nc.vector.tensor_reduce(out=r[:, 0:1], in_=eimpP[:], axis=AX.X,
                                    op=OP.add)
            gat = gaP[:].rearrange("p (b a) -> p a b", a=4)
            nc.vector.tensor_reduce(
                out=r[:, 1:5].rearrange("p (a o) -> p a o", o=1),
                in_=gat, axis=AX.X, op=OP.add)
            nc.vector.tensor_reduce(out=r[:, 5:6], in_=pvP[:], axis=AX.X,
                                    op=OP.add)
            nc.vector.tensor_reduce(out=r[:, 6:7], in_=cvvP[:], axis=AX.X,
                                    op=OP.add)
            nc.vector.tensor_reduce(out=r[:, 7:8], in_=validP[:], axis=AX.X,
                                    op=OP.add)
            bvt = bvP[:].rearrange("p (b a) -> p a b", a=4)
            nc.vector.tensor_reduce(
                out=r[:, 8:12].rearrange("p (a o) -> p a o", o=1),
                in_=bvt, axis=AX.X, op=OP.add)
            orgp = ppm.tile([32, 512], F32, tag="mmp")
            nc.tensor.matmul(out=orgp[0:1, 0:12], lhsT=ones[:], rhs=r[:],
                             start=True, stop=True)
            orgs = bp.tile([1, 12], F32, tag="orgs")
            nc.vector.tensor_copy(out=orgs[:], in_=orgp[0:1, 0:12])
            nc.sync.dma_start(out=out_org[:], in_=orgs[:])
    return nc


_NC_CACHE = None


def _get_program():
    global _NC_CACHE
    if _NC_CACHE is None:
        _NC_CACHE = build_program()
    return _NC_CACHE


def _host_prep_core(c, state, arch, eph, surprise, seg_ids):
    B0 = int(np.searchsorted(seg_ids, 2048 * c))
    B1 = int(np.searchsorted(seg_ids, 2048 * (c + 1)))
    Nc = B1 - B0
    lseg = (seg_ids[B0:B1] - 2048 * c).astype(np.int64)
    idx = np.full(NPAD, -1, np.int64)
    rel = np.full(NPAD, PADSEG, np.float32)
    cur = 0
    for S in range(NTILES):
        blk = S // TPB
        f = _window_start(S)
        wlo = 512 * blk + f
        whi = wlo + W
        take = min(128, int(np.searchsorted(lseg, whi)) - cur)
        if take > 0:
            assert lseg[cur] >= wlo, f"core {c} tile {S}: behind-lag"
            sl = np.arange(cur, cur + take)
            idx[S * 128:S * 128 + take] = sl
            rel[S * 128:S * 128 + take] = (lseg[sl] - wlo).astype(np.float32)
            cur += take
    assert cur == Nc, f"core {c}: {Nc - cur} cells not scheduled"
    m = idx >= 0

    def g(x):
        out = np.zeros((NPAD,) + x.shape[1:], np.float32)
        out[m] = x[B0:B1][idx[m]]
        return out

    return g(state), g(arch), g(eph), g(surprise), rel


def _swz1(x):
    return x.reshape(NCHUNK, 2, 32, 128).transpose(0, 3, 2, 1).reshape(NCHUNK, 128, 64)


def kernel(state, arch, energy, phi_local, surprise, seg_ids, n_clusters,
           W1, b1, W2, b2, V1, c1, V2, c2):
    state = np.asarray(state, np.float32)
    arch = np.asarray(arch, np.float32)
    energy = np.asarray(energy, np.float32)
    phi_local = np.asarray(phi_local, np.float32)
    surprise = np.asarray(surprise, np.float32)
    seg_ids = np.asarray(seg_ids)
    W1 = np.asarray(W1, np.float32); b1 = np.asarray(b1, np.float32)
    W2 = np.asarray(W2, np.float32); b2 = np.asarray(b2, np.float32)
    V1 = np.asarray(V1, np.float32); c1 = np.asarray(c1, np.float32)
    V2 = np.asarray(V2, np.float32); c2 = np.asarray(c2, np.float32)

    w1d = np.zeros((72, 128), np.float32)
    w1d[0:36, 0:64] = W1
    w1d[36:72, 64:128] = W1
    w2d = np.zeros((128, 2), np.float32)
    w2d[0:64, 0] = W2[:, 0]
    w2d[64:128, 1] = W2[:, 0]
    consts = dict(
        w1d=w1d.astype(ml_dtypes.bfloat16),
        b1d=np.concatenate([b1, b1]).reshape(128, 1).astype(np.float32),
        w2d=w2d.astype(ml_dtypes.bfloat16),
        b2n=np.full((128, 1), 0.5 * b2[0], np.float32),
        v1=V1, c1b=c1.reshape(32, 1), v2=V2,
        c2n=np.full((128, 1), 0.5 * c2[0], np.float32),
    )
    eph_full = (energy * phi_local).astype(np.float32)
    in_maps = []
    for c in range(NCORES):
        st, ar, ep, su, rel = _host_prep_core(
            c, state, arch, eph_full, surprise, seg_ids)
        f36 = np.concatenate([st.T, ar.T], 0)              # [36, NPAD]
        featsT = f36.reshape(36, NCHUNK, 2, 4096).transpose(2, 0, 1, 3).reshape(
            72, NPAD // 2).astype(ml_dtypes.bfloat16)
        # vtstat [NCHUNK, 128, 64, 15]: lanes 0=1, 1=0(ew), 2:6=0(ew*a),
        # 6:10=a, 10:14=a*a, 14=sur
        acm = ar.reshape(NCHUNK, 2, 32, 128, 4).transpose(0, 3, 2, 1, 4).reshape(
            NCHUNK, 128, 64, 4)
        vst = np.zeros((NCHUNK, 128, 64, 15), np.float32)
        vst[..., 0] = 1.0
        vst[..., 6:10] = acm
        vst[..., 10:14] = acm * acm
        vst[..., 14] = _swz1(su)
        relz = _swz1(rel)  # [NCHUNK, 128, 64]
        ohv = (relz[..., None] == np.arange(W, dtype=np.float32)).astype(
            ml_dtypes.float8_e4m3)
        epz = _swz1(ep)
        cvv = np.concatenate([epz, 0.5 * epz], axis=2)
        in_maps.append(dict(
            featsT=np.ascontiguousarray(featsT),
            vtstat=np.ascontiguousarray(
                vst.reshape(NCHUNK, 128, 960).astype(ml_dtypes.bfloat16)),
            cellvec=np.ascontiguousarray(cvv.astype(ml_dtypes.bfloat16)),
            ohdram=np.ascontiguousarray(ohv.reshape(NCHUNK, 128, 2048)),
            **consts))
    nc = _get_program()
    res = run_bass_kernel_spmd(nc, in_maps, list(range(NCORES)))
    outs = res.results
    couts = [np.asarray(outs[c]["out_cluster"]) for c in range(NCORES)]
    orgs = [np.asarray(outs[c]["out_org"]).reshape(12) for c in range(NCORES)]
    cluster_full = np.concatenate(couts, 0).astype(np.float32)
    p = np.sum(np.stack(orgs, 0), 0, dtype=np.float64)
    Z, G, sphi, scoh, nval, pres = p[0], p[1:5], p[5], p[6], p[7], p[8:12]
    ga = (G / Z).astype(np.float32)
    e = np.exp(ga - ga.max())
    global_arch = (e / e.sum()).astype(np.float32)
    n_valid = max(nval, 1.0)
    avg_phi = sphi / n_valid
    unique = float((pres > 0).sum())
    phi_global = min(1.0, avg_phi * (0.5 + 0.5 * unique / 4.0))
    vert = scoh / n_valid
    self_model = np.array([*global_arch, phi_global, vert], np.float32)
    return np.concatenate([cluster_full.reshape(-1), self_model]).astype(np.float32)


# revision 37
# speedup vs baseline: 1.0914x; 1.0914x over previous
"""Trainium2 Bass kernel for nn_BottomUpIntegrator (gnn_message_passing).

Sharding: cells split at cluster boundaries across 8 cores (2048 clusters each);
per-core segment sums via one-hot scatter matmuls into PSUM with a core-invariant
static window schedule; cluster+organism phase on-chip; host combines 12 organism
partial floats per core into the final 6 self-model outputs.

v4: single ACT table set (sigmoid via tanh), relu split ACT/DVE, one-hot shipped
from host as fp8 (exact 0/1; PE takes bf16 weights x fp8 moving), static scatter
values precomputed on host, scatter software-pipelined one chunk behind, single
PSUM bank for all 4 scatter accumulators (32-aligned partition groups), and the
cluster phase (phase B) executed per 512-cluster block pipelined INSIDE phase A:
all its transposes run on DVE StreamTranspose (32x32 blocks), the cluster-MLP
second layer lands cluster-major via activations-as-stationary matmuls, so only
the organism reductions remain in the tail.
"""
import numpy as np
import ml_dtypes

import json as _json

from concourse import bass, mybir
from concourse import bass2jax as _b2j
from concourse import bass_utils as _bu
from concourse.tile import TileContext
from concourse.bass_utils import run_bass_kernel_spmd

_orig_compile = _bu.compile_bir_kernel


def _split_waits_compile(bir_json, tmpdir, neff_name="file.neff"):
    """Walrus lowers at most ONE semaphore wait per TPB instruction struct.
    Tile emits several. Hoist extras onto injected same-engine EventSemaphore
    wait instructions immediately before the owner (semantically identical:
    engines execute in program order)."""
    d = _json.loads(bir_json)
    cnt = 0
    for fn in d["functions"]:
        for blk in fn["blocks"]:
            newlist = []
            for ins in blk["instructions"]:
                si = ins.get("sync_info")
                waits = si.get("on_wait", []) if si else []
                if si and len(waits) > 1 and ins.get("opcode") not in (
                        "EventSemaphore",):
                    for w_i, w in enumerate(waits[:-1]):
                        cnt += 1
                        newlist.append({
                            "debug": ins.get("debug", 0),
                            "engine": ins["engine"],
                            "ins": [], "outs": [],
                            "name": f"{ins['name']}-wsplit{w_i}",
                            "opcode": "EventSemaphore",
                            "sync_info": {"on_update": [], "on_wait": [w]},
                        })
                    si["on_wait"] = [waits[-1]]
                newlist.append(ins)
            blk["instructions"] = newlist
    print(f"[wait-split] hoisted {cnt} extra waits")
    return _orig_compile(_json.dumps(d).encode(), tmpdir, neff_name=neff_name)


_bu.compile_bir_kernel = _split_waits_compile
_b2j.compile_bir_kernel = _split_waits_compile

F32 = mybir.dt.float32
BF16 = mybir.dt.bfloat16
FP8 = mybir.dt.float8e4
AF = mybir.ActivationFunctionType
OP = mybir.AluOpType
AX = mybir.AxisListType

NCORES = 8
KLOC = 2048            # clusters per core
NPAD = 262144          # padded cells per core
CHUNK = 8192           # cells per chunk
NCHUNK = NPAD // CHUNK # 32
W = 32                 # onehot window width (clusters)
NTILES = NPAD // 128   # 2048 scatter tiles per core
TPB = NTILES // 4      # tiles per 512-cluster block
PADSEG = 1.0e9
DVE_RELU = (2, 5, 7)   # relu tiles evacuated on DVE instead of ACT


def _window_start(S):
    s = S % TPB
    return int(np.clip(s - 16, 0, 512 - W))


def build_program():
    nc = bass.Bass(trn_type="TRN2", use_seq_codegen=True)
    featsT = nc.dram_tensor("featsT", [72, NPAD // 2], BF16, kind="ExternalInput")
    vtstat = nc.dram_tensor("vtstat", [NCHUNK, 128, 960], BF16, kind="ExternalInput")
    cellvec = nc.dram_tensor("cellvec", [NCHUNK, 128, 128], BF16, kind="ExternalInput")
    ohdram = nc.dram_tensor("ohdram", [NCHUNK, 128, 2048], FP8, kind="ExternalInput")
    w1d = nc.dram_tensor("w1d", [72, 128], BF16, kind="ExternalInput")
    b1d = nc.dram_tensor("b1d", [128, 1], F32, kind="ExternalInput")
    w2d = nc.dram_tensor("w2d", [128, 2], BF16, kind="ExternalInput")
    b2n = nc.dram_tensor("b2n", [128, 1], F32, kind="ExternalInput")
    v1 = nc.dram_tensor("v1", [7, 32], F32, kind="ExternalInput")
    c1b = nc.dram_tensor("c1b", [32, 1], F32, kind="ExternalInput")
    v2 = nc.dram_tensor("v2", [32, 1], F32, kind="ExternalInput")
    c2n = nc.dram_tensor("c2n", [128, 1], F32, kind="ExternalInput")
    out_cluster = nc.dram_tensor("out_cluster", [KLOC, 8], F32, kind="ExternalOutput")
    out_org = nc.dram_tensor("out_org", [1, 12], F32, kind="ExternalOutput")

    with TileContext(nc) as tc:
        with (
            tc.tile_pool(name="const", bufs=1) as cp,
            tc.tile_pool(name="feats", bufs=3) as fp,
            tc.tile_pool(name="hs", bufs=2) as hp,
            tc.tile_pool(name="small", bufs=4) as sp,
            tc.tile_pool(name="scatv", bufs=3) as vp,
            tc.tile_pool(name="ph_b", bufs=1) as bp,
            tc.tile_pool(name="scatps", bufs=1, space="PSUM") as pps,
            tc.tile_pool(name="mm1ps", bufs=5, space="PSUM") as pp1,
            tc.tile_pool(name="mm2ps", bufs=1, space="PSUM") as pp2,
            tc.tile_pool(name="mmbps", bufs=1, space="PSUM") as ppm,
        ):
            # ---- constants ----------------------------------------------
            w1s = cp.tile([72, 128], BF16, tag="w1s")
            nc.sync.dma_start(out=w1s[:], in_=w1d[:])
            b1s = cp.tile([128, 1], F32, tag="b1s")
            nc.sync.dma_start(out=b1s[:], in_=b1d[:])
            w2s = cp.tile([128, 2], BF16, tag="w2s")
            nc.sync.dma_start(out=w2s[:], in_=w2d[:])
            b2ns = cp.tile([128, 1], F32, tag="b2ns")
            nc.sync.dma_start(out=b2ns[:], in_=b2n[:])
            v1s = cp.tile([7, 32], F32, tag="v1s")
            nc.sync.dma_start(out=v1s[:], in_=v1[:])
            c1s = cp.tile([32, 1], F32, tag="c1s")
            nc.sync.dma_start(out=c1s[:], in_=c1b[:])
            v2s = cp.tile([32, 1], F32, tag="v2s")
            nc.sync.dma_start(out=v2s[:], in_=v2[:])
            c2ns = cp.tile([128, 1], F32, tag="c2ns")
            nc.sync.dma_start(out=c2ns[:], in_=c2n[:])
            ones = cp.tile([128, 1], F32, tag="ones")
            nc.vector.memset(ones[:], 1.0)

            zbf = cp.tile([128, 512], BF16, tag="zbf")
            nc.vector.memset(zbf[:], 0.0)

            # Pre-touch DMA-loaded constants on their consuming engines so no
            # later compute instruction needs a second (DMA) semaphore wait.
            scra = cp.tile([128, 4], F32, tag="scra")
            nc.scalar.activation(out=scra[:, 0:1], in_=b1s[:], func=AF.Copy)
            nc.scalar.activation(out=scra[:, 1:2], in_=b2ns[:], func=AF.Copy)
            nc.scalar.activation(out=scra[0:32, 2:3], in_=c1s[:], func=AF.Copy)
            nc.scalar.activation(out=scra[:, 3:4], in_=c2ns[:], func=AF.Copy)
            scrv = cp.tile([1, 1], F32, tag="scrv")
            nc.vector.tensor_copy(out=scrv[:], in_=b1s[0:1, 0:1])

            # persistent scatter accumulators: ONE PSUM bank; block b's
            # [15, 512] accumulator lives at partitions 32b..32b+15.
            scatb = pps.tile([128, 512], F32, tag="scatb", name="scatb")
            scat = [scatb[32 * b:32 * b + 15, :] for b in range(4)]
            # PE touch of w1s (rides on Ldweights; result overwritten below).
            nc.tensor.matmul(out=scatb[0:1, 0:1], lhsT=w1s[0:1, 0:1],
                             rhs=zbf[0:1, 0:1], start=True, stop=True,
                             skip_group_check=True)
            nc.tensor.matmul(out=scatb[:], lhsT=zbf[:, 0:128], rhs=zbf[:],
                             start=True, stop=False, skip_group_check=True)

            # ---- persistent phase-B tiles -------------------------------
            tt32 = cp.tile([128, 512], F32, tag="tt32")    # [p, ct*32+v]
            cftt32 = cp.tile([32, 2048], F32, tag="cftt32")  # [q(7 used), cluster]
            aggP = cp.tile([128, 64], F32, tag="aggP")
            gaP = cp.tile([128, 64], F32, tag="gaP")
            bvP = cp.tile([128, 64], F32, tag="bvP")
            phicP = cp.tile([128, 16], F32, tag="phicP")
            cohP = cp.tile([128, 16], F32, tag="cohP")
            perrP = cp.tile([128, 16], F32, tag="perrP")
            integP = cp.tile([128, 16], F32, tag="integP")
            validP = cp.tile([128, 16], F32, tag="validP")
            eimpP = cp.tile([128, 16], F32, tag="eimpP")
            basecP = cp.tile([128, 16], F32, tag="basecP")
            pvP = cp.tile([128, 16], F32, tag="pvP")
            cvvP = cp.tile([128, 16], F32, tag="cvvP")

            pbstate = {}

            def lt(tag, w=4):
                return bp.tile([128, w], F32, tag=tag, name=tag)

            def pb_part0(b):
                # tt transposes straight from the PSUM accumulator bank
                # (rows 32b+15..32b+31 were only touched by the zeroing
                # matmul, so the extra 17 rows transpose in as zeros).
                for t in range(4):
                    ct = 4 * b + t
                    for i in range(4):
                        o = 128 * t + 32 * i
                        nc.vector.transpose(
                            out=tt32[32 * i:32 * i + 32, 32 * ct:32 * ct + 32],
                            in_=scatb[32 * b:32 * b + 32, o:o + 32])

            def pb_part1(b):
                # per-cluster stats for this block's 4 cluster-tiles
                B4 = slice(4 * b, 4 * (b + 1))
                B16 = slice(16 * b, 16 * (b + 1))
                tv = tt32[:, 128 * b:128 * (b + 1)].rearrange(
                    "p (t v) -> p t v", v=32)
                cnt = tv[:, :, 0:1]
                sew = tv[:, :, 1:2]
                sewa = tv[:, :, 2:6]
                sa = tv[:, :, 6:10]
                ssq = tv[:, :, 10:14]
                ssur = tv[:, :, 14:15]

                def v3(t):
                    return t[:].rearrange("p (b a) -> p b a", a=1)

                def v44(t):
                    return t[:].rearrange("p (b a) -> p b a", a=4)

                cntc = lt("cntc")
                nc.vector.tensor_scalar(out=v3(cntc), in0=cnt, scalar1=1.0,
                                        scalar2=None, op0=OP.max)
                rc = lt("rc")
                nc.vector.reciprocal(out=rc[:], in_=cntc[:])
                den = lt("den")
                nc.vector.tensor_scalar(out=v3(den), in0=sew, scalar1=1.0,
                                        scalar2=None, op0=OP.max)
                rden = lt("rden")
                nc.vector.reciprocal(out=rden[:], in_=den[:])
                agr = lt("agr", 16)
                nc.vector.tensor_tensor(out=v44(agr), in0=sewa,
                                        in1=rden[:].to_broadcast([128, 4, 4]),
                                        op=OP.mult)
                mx = lt("mx")
                nc.vector.tensor_reduce(out=v3(mx), in_=v44(agr), axis=AX.X,
                                        op=OP.max)
                es = lt("es", 16)
                nc.vector.tensor_tensor(out=v44(es), in0=v44(agr),
                                        in1=mx[:].to_broadcast([128, 4, 4]),
                                        op=OP.subtract)
                nc.scalar.activation(out=es[:], in_=es[:], func=AF.Exp)
                ssum = lt("ssum")
                nc.vector.tensor_reduce(out=v3(ssum), in_=v44(es), axis=AX.X,
                                        op=OP.add)
                rssum = lt("rssum")
                nc.vector.reciprocal(out=rssum[:], in_=ssum[:])
                aggv = aggP[:, B16].rearrange("p (b a) -> p b a", a=4)
                nc.vector.tensor_tensor(out=aggv, in0=v44(es),
                                        in1=rssum[:].to_broadcast([128, 4, 4]),
                                        op=OP.mult)
                mean = lt("mean", 16)
                nc.vector.tensor_tensor(out=v44(mean), in0=sa,
                                        in1=rc[:].to_broadcast([128, 4, 4]),
                                        op=OP.mult)
                var = lt("var", 16)
                nc.vector.tensor_tensor(out=v44(var), in0=v44(mean),
                                        in1=v44(mean), op=OP.mult)
                cntb = lt("cntb")
                nc.vector.tensor_copy(out=v3(cntb), in_=cnt)
                nc.vector.tensor_tensor(out=v44(var), in0=v44(var),
                                        in1=cntb[:].to_broadcast([128, 4, 4]),
                                        op=OP.mult)
                nc.vector.tensor_tensor(out=v44(var), in0=ssq, in1=v44(var),
                                        op=OP.subtract)
                cm1 = lt("cm1")
                nc.vector.tensor_scalar(out=v3(cm1), in0=cnt, scalar1=-1.0,
                                        scalar2=1.0, op0=OP.add, op1=OP.max)
                rcm1 = lt("rcm1")
                nc.vector.reciprocal(out=rcm1[:], in_=cm1[:])
                nc.vector.tensor_tensor(out=v44(var), in0=v44(var),
                                        in1=rcm1[:].to_broadcast([128, 4, 4]),
                                        op=OP.mult)
                vm = lt("vm")
                nc.vector.tensor_reduce(out=v3(vm), in_=v44(var), axis=AX.X,
                                        op=OP.add)
                nc.vector.tensor_scalar(out=vm[:], in0=vm[:], scalar1=0.25,
                                        scalar2=None, op0=OP.mult)
                phs = phicP[:, B4]
                nc.vector.tensor_scalar(out=phs, in0=vm[:], scalar1=2.0,
                                        scalar2=1.0, op0=OP.mult, op1=OP.min)
                nc.vector.tensor_scalar(out=phs, in0=phs, scalar1=-1.0,
                                        scalar2=1.0, op0=OP.mult, op1=OP.add)
                nc.vector.tensor_scalar(out=cohP[:, B4], in0=vm[:], scalar1=-1.0,
                                        scalar2=1.0, op0=OP.mult, op1=OP.add)
                nc.vector.tensor_tensor(
                    out=perrP[:, B4].rearrange("p (b a) -> p b a", a=1),
                    in0=ssur, in1=v3(rc), op=OP.mult)
                igs = integP[:, B4]
                nc.vector.tensor_scalar(out=igs, in0=perrP[:, B4], scalar1=-1.0,
                                        scalar2=1.0, op0=OP.mult, op1=OP.add)
                nc.vector.tensor_tensor(out=igs, in0=igs, in1=phs, op=OP.mult)
                nc.vector.tensor_scalar(
                    out=validP[:, B4].rearrange("p (b a) -> p b a", a=1),
                    in0=cnt, scalar1=0.0, scalar2=None, op0=OP.is_gt)
                szf = lt("szf")
                nc.vector.tensor_scalar(out=v3(szf), in0=cnt, scalar1=0.05,
                                        scalar2=1.0, op0=OP.mult, op1=OP.min)
                # cluster features, 32-padded per cluster-tile, then transpose
                cft32 = lt("cft32", 128)
                cfv = cft32[:].rearrange("p (t q) -> p t q", q=32)
                nc.gpsimd.tensor_copy(out=cfv[:, :, 0:4], in_=aggv)
                nc.gpsimd.tensor_copy(out=cfv[:, :, 4:5],
                                      in_=phs.to_broadcast([128, 4, 1]))
                nc.gpsimd.tensor_copy(out=cfv[:, :, 5:6],
                                      in_=cohP[:, B4].to_broadcast([128, 4, 1]))
                nc.gpsimd.tensor_copy(out=cfv[:, :, 6:7],
                                      in_=szf[:].to_broadcast([128, 4, 1]))
                for t in range(4):
                    for i in range(4):
                        nc.vector.transpose(
                            out=cftt32[0:32, 512 * b + 128 * t + 32 * i:
                                       512 * b + 128 * t + 32 * i + 32],
                            in_=cft32[32 * i:32 * i + 32, 32 * t:32 * t + 32])

            def pb_part2(b):
                B4 = slice(4 * b, 4 * (b + 1))
                B16 = slice(16 * b, 16 * (b + 1))
                aggv = aggP[:, B16].rearrange("p (b a) -> p b a", a=4)
                # cluster MLP: layer 1 cluster-free-major, layer 2 back to
                # cluster-partition-major via activations-as-stationary
                hcp = ppm.tile([32, 512], F32, tag="mmp")
                nc.tensor.matmul(out=hcp[:], lhsT=v1s[:],
                                 rhs=cftt32[0:7, 512 * b:512 * (b + 1)],
                                 start=True, stop=True)
                hcs = bp.tile([32, 512], F32, tag="hcs", name="hcs")
                nc.scalar.activation(out=hcs[:], in_=hcp[:], func=AF.Relu,
                                     bias=c1s[:])
                bcp4 = pp1.tile([128, 512], F32, tag="hp1")
                for t in range(4):
                    nc.tensor.matmul(out=bcp4[:, t:t + 1],
                                     lhsT=hcs[:, 128 * t:128 * (t + 1)],
                                     rhs=v2s[:], start=True, stop=True,
                                     skip_group_check=True)
                bsl = basecP[:, B4]
                nc.scalar.activation(out=bsl, in_=bcp4[:, 0:4], func=AF.Tanh,
                                     bias=c2ns[:], scale=0.5)
                nc.vector.tensor_scalar(out=bsl, in0=bsl, scalar1=1.0,
                                        scalar2=0.5, op0=OP.add, op1=OP.mult)
                impc = lt("impcB")
                nc.vector.tensor_tensor(out=impc[:], in0=bsl, in1=phicP[:, B4],
                                        op=OP.mult)
                nc.vector.tensor_scalar(out=impc[:], in0=impc[:], scalar1=0.01,
                                        scalar2=1.0, op0=OP.max, op1=OP.min)
                eimp0 = lt("eimp0")
                nc.scalar.activation(out=eimp0[:], in_=impc[:], func=AF.Exp)
                nc.vector.tensor_tensor(out=eimpP[:, B4], in0=eimp0[:],
                                        in1=validP[:, B4], op=OP.mult)
                # specialization one-hot with first-match-wins
                amx = lt("amx")
                nc.vector.tensor_reduce(
                    out=amx[:].rearrange("p (b a) -> p b a", a=1),
                    in_=aggv, axis=AX.X, op=OP.max)
                bselv = bvP[:, B16].rearrange("p (b a) -> p b a", a=4)
                nc.vector.tensor_tensor(out=bselv, in0=aggv,
                                        in1=amx[:].to_broadcast([128, 4, 4]),
                                        op=OP.is_equal)
                taken = lt("taken")
                nc.vector.memset(taken[:], 0.0)
                notk = lt("notk")
                for a in range(4):
                    nc.vector.tensor_scalar(out=notk[:], in0=taken[:],
                                            scalar1=-1.0, scalar2=1.0,
                                            op0=OP.mult, op1=OP.add)
                    nc.vector.tensor_tensor(
                        out=bselv[:, :, a:a + 1], in0=bselv[:, :, a:a + 1],
                        in1=notk[:].rearrange("p (b a) -> p b a", a=1),
                        op=OP.mult)
                    if a < 3:
                        nc.vector.tensor_tensor(
                            out=taken[:].rearrange("p (b a) -> p b a", a=1),
                            in0=taken[:].rearrange("p (b a) -> p b a", a=1),
                            in1=bselv[:, :, a:a + 1], op=OP.max)
                nc.vector.tensor_tensor(out=bselv, in0=bselv,
                                        in1=validP[:, B4].to_broadcast([128, 4, 4]),
                                        op=OP.mult)
                nc.vector.tensor_tensor(out=gaP[:, B16].rearrange(
                    "p (b a) -> p b a", a=4), in0=aggv,
                    in1=eimpP[:, B4].to_broadcast([128, 4, 4]), op=OP.mult)
                nc.vector.tensor_tensor(out=pvP[:, B4], in0=phicP[:, B4],
                                        in1=validP[:, B4], op=OP.mult)
                nc.vector.tensor_tensor(out=cvvP[:, B4], in0=cohP[:, B4],
                                        in1=validP[:, B4], op=OP.mult)
                # cluster_out writeback for this block
                oc32 = lt("oc32", 32)
                ocv = oc32[:].rearrange("p (t q) -> p t q", q=8)
                nc.gpsimd.tensor_copy(out=ocv[:, :, 0:4], in_=aggv)
                nc.gpsimd.tensor_copy(out=ocv[:, :, 4:5],
                                      in_=phicP[:, B4].to_broadcast([128, 4, 1]))
                nc.gpsimd.tensor_copy(out=ocv[:, :, 5:6],
                                      in_=cohP[:, B4].to_broadcast([128, 4, 1]))
                nc.gpsimd.tensor_copy(out=ocv[:, :, 6:7],
                                      in_=perrP[:, B4].to_broadcast([128, 4, 1]))
                nc.gpsimd.tensor_copy(out=ocv[:, :, 7:8],
                                      in_=integP[:, B4].to_broadcast([128, 4, 1]))
                nc.sync.dma_start(
                    out=out_cluster[:].rearrange("(c p) q -> p c q", p=128)[
                        :, 4 * b:4 * (b + 1), :],
                    in_=ocv)

            PB_PARTS = (pb_part0, pb_part1, pb_part2)

            def emit_scatter(k, vts, oh):
                # scatter: col j -> sorted tile S = 64k + 32*(j%2) + j//2
                for j in range(64):
                    S = 64 * k + 32 * (j % 2) + (j // 2)
                    blk = S // TPB
                    f = _window_start(S)
                    nc.tensor.matmul(out=scat[blk][:, f:f + W],
                                     lhsT=vts[:, 15 * j:15 * j + 15],
                                     rhs=oh[:, 32 * j:32 * j + 32],
                                     start=False,
                                     stop=(k == NCHUNK - 1 and j >= 62),
                                     skip_group_check=True,
                                     tile_position=(0, 32 * blk))

            # ---- phase A ------------------------------------------------
            prev = None
            for k in range(NCHUNK):
                ft = fp.tile([72, 4096], BF16, tag="ft")
                nc.sync.dma_start(out=ft[:],
                                  in_=featsT[:, k * 4096:(k + 1) * 4096])
                vts = vp.tile([128, 960], BF16, tag="vts")
                nc.sync.dma_start(out=vts[:], in_=vtstat[k])
                cv = sp.tile([128, 128], BF16, tag="cv")
                nc.sync.dma_start(out=cv[:], in_=cellvec[k])
                oh = vp.tile([128, 2048], FP8, tag="oh")
                nc.sync.dma_start(out=oh[:], in_=ohdram[k])

                # mm1 + bias + relu -> h [128, 4096] bf16 (ACT/DVE split;
                # lighter DVE share on iterations that also carry a pipelined
                # phase-B part)
                dve_relu = (7,) if (k >= 8 and k % 8 in (0, 1, 2)) else DVE_RELU
                hs = hp.tile([128, 4096], BF16, tag="hs")
                for j in range(8):
                    hp1 = pp1.tile([128, 512], F32, tag="hp1")
                    nc.tensor.matmul(out=hp1[:], lhsT=w1s[:],
                                     rhs=ft[:, 512 * j:512 * (j + 1)],
                                     start=True, stop=True)
                    dst = hs[:, 512 * j:512 * (j + 1)]
                    if j in dve_relu:
                        nc.vector.tensor_scalar(
                            out=dst, in0=hp1[:], scalar1=b1s[:],
                            scalar2=0.0, op0=OP.add, op1=OP.max)
                    else:
                        nc.scalar.activation(out=dst, in_=hp1[:],
                                             func=AF.Relu, bias=b1s[:])

                # mm2 -> base logits [128, 64] cell-major in PSUM
                bb = pp2.tile([128, 64], F32, tag="bb")
                for t in range(32):
                    nc.tensor.matmul(out=bb[:, 2 * t:2 * t + 2],
                                     lhsT=hs[:, 128 * t:128 * (t + 1)], rhs=w2s[:],
                                     start=(t == 0), stop=(t == 31),
                                     skip_group_check=True)

                # sigmoid via tanh: sig = (1 + tanh((bb+b2)/2)) / 2, so
                # sig*eph = (th + 1) * (eph/2) -- one STT op.
                # cv lanes: 0:64 eph, 64:128 eph/2
                th = sp.tile([128, 64], F32, tag="th")
                nc.scalar.activation(out=th[:], in_=bb[:], func=AF.Tanh,
                                     bias=b2ns[:], scale=0.5)
                imp = sp.tile([128, 64], F32, tag="imp")
                nc.vector.scalar_tensor_tensor(
                    out=imp[:], in0=th[:], scalar=1.0, in1=cv[:, 64:128],
                    op0=OP.add, op1=OP.mult)
                impc = sp.tile([128, 64], F32, tag="impc")
                nc.vector.tensor_scalar(out=impc[:], in0=imp[:], scalar1=0.01,
                                        scalar2=1.0, op0=OP.max, op1=OP.min)
                wc = sp.tile([128, 64], F32, tag="wc")
                nc.vector.tensor_tensor(out=wc[:], in0=impc[:], in1=cv[:, 0:64],
                                        op=OP.mult)
                ew = sp.tile([128, 64], BF16, tag="ew")
                nc.scalar.activation(out=ew[:], in_=wc[:], func=AF.Exp)

                # fill dynamic lanes of vt: [p, s, 15] lanes 1 (ew), 2:6 (ew*a)
                vv = vts[:].rearrange("p (s v) -> p s v", v=15)
                nc.vector.tensor_copy(out=vv[:, :, 1:2],
                                      in_=ew[:].to_broadcast([128, 64, 1]))
                nc.vector.tensor_tensor(out=vv[:, :, 2:6], in0=vv[:, :, 6:10],
                                        in1=ew[:].to_broadcast([128, 64, 4]),
                                        op=OP.mult)

                # scatter of the PREVIOUS chunk (software pipelining: its
                # vt/oh are long ready, so the PE never stalls mid-chunk)
                if prev is not None:
                    emit_scatter(*prev)
                prev = (k, vts, oh)

                # pipelined phase-B parts: block b's parts at k=8b+8..8b+10
                if k >= 8 and k % 8 in (0, 1, 2):
                    PB_PARTS[k % 8](k // 8 - 1)

            emit_scatter(*prev)
            for part in PB_PARTS:
                part(3)

            # ---- organism tail ------------------------------------------
            r = bp.tile([128, 12], F32, tag="r")
            nc.vector.tensor_reduce(out=r[:, 0:1], in_=eimpP[:], axis=AX.X,
                                    op=OP.add)
            gat = gaP[:].rearrange("p (b a) -> p a b", a=4)
            nc.vector.tensor_reduce(
                out=r[:, 1:5].rearrange("p (a o) -> p a o", o=1),
                in_=gat, axis=AX.X, op=OP.add)
            nc.vector.tensor_reduce(out=r[:, 5:6], in_=pvP[:], axis=AX.X,
                                    op=OP.add)
            nc.vector.tensor_reduce(out=r[:, 6:7], in_=cvvP[:], axis=AX.X,
                                    op=OP.add)
            nc.vector.tensor_reduce(out=r[:, 7:8], in_=validP[:], axis=AX.X,
                                    op=OP.add)
            bvt = bvP[:].rearrange("p (b a) -> p a b", a=4)
            nc.vector.tensor_reduce(
                out=r[:, 8:12].rearrange("p (a o) -> p a o", o=1),
                in_=bvt, axis=AX.X, op=OP.add)
            orgp = ppm.tile([32, 512], F32, tag="mmp")
            nc.tensor.matmul(out=orgp[0:1, 0:12], lhsT=ones[:], rhs=r[:],
                             start=True, stop=True)
            orgs = bp.tile([1, 12], F32, tag="orgs")
            nc.vector.tensor_copy(out=orgs[:], in_=orgp[0:1, 0:12])
            nc.sync.dma_start(out=out_org[:], in_=orgs[:])
    return nc


_NC_CACHE = None


def _get_program():
    global _NC_CACHE
    if _NC_CACHE is None:
        _NC_CACHE = build_program()
    return _NC_CACHE


def _host_prep_core(c, state, arch, eph, surprise, seg_ids):
    B0 = int(np.searchsorted(seg_ids, 2048 * c))
    B1 = int(np.searchsorted(seg_ids, 2048 * (c + 1)))
    Nc = B1 - B0
    lseg = (seg_ids[B0:B1] - 2048 * c).astype(np.int64)
    idx = np.full(NPAD, -1, np.int64)
    rel = np.full(NPAD, PADSEG, np.float32)
    cur = 0
    for S in range(NTILES):
        blk = S // TPB
        f = _window_start(S)
        wlo = 512 * blk + f
        whi = wlo + W
        take = min(128, int(np.searchsorted(lseg, whi)) - cur)
        if take > 0:
            assert lseg[cur] >= wlo, f"core {c} tile {S}: behind-lag"
            sl = np.arange(cur, cur + take)
            idx[S * 128:S * 128 + take] = sl
            rel[S * 128:S * 128 + take] = (lseg[sl] - wlo).astype(np.float32)
            cur += take
    assert cur == Nc, f"core {c}: {Nc - cur} cells not scheduled"
    m = idx >= 0

    def g(x):
        out = np.zeros((NPAD,) + x.shape[1:], np.float32)
        out[m] = x[B0:B1][idx[m]]
        return out

    return g(state), g(arch), g(eph), g(surprise), rel


def _swz1(x):
    return x.reshape(NCHUNK, 2, 32, 128).transpose(0, 3, 2, 1).reshape(NCHUNK, 128, 64)


def kernel(state, arch, energy, phi_local, surprise, seg_ids, n_clusters,
           W1, b1, W2, b2, V1, c1, V2, c2):
    state = np.asarray(state, np.float32)
    arch = np.asarray(arch, np.float32)
    energy = np.asarray(energy, np.float32)
    phi_local = np.asarray(phi_local, np.float32)
    surprise = np.asarray(surprise, np.float32)
    seg_ids = np.asarray(seg_ids)
    W1 = np.asarray(W1, np.float32); b1 = np.asarray(b1, np.float32)
    W2 = np.asarray(W2, np.float32); b2 = np.asarray(b2, np.float32)
    V1 = np.asarray(V1, np.float32); c1 = np.asarray(c1, np.float32)
    V2 = np.asarray(V2, np.float32); c2 = np.asarray(c2, np.float32)

    w1d = np.zeros((72, 128), np.float32)
    w1d[0:36, 0:64] = W1
    w1d[36:72, 64:128] = W1
    w2d = np.zeros((128, 2), np.float32)
    w2d[0:64, 0] = W2[:, 0]
    w2d[64:128, 1] = W2[:, 0]
    consts = dict(
        w1d=w1d.astype(ml_dtypes.bfloat16),
        b1d=np.concatenate([b1, b1]).reshape(128, 1).astype(np.float32),
        w2d=w2d.astype(ml_dtypes.bfloat16),
        b2n=np.full((128, 1), 0.5 * b2[0], np.float32),
        v1=V1, c1b=c1.reshape(32, 1), v2=V2,
        c2n=np.full((128, 1), 0.5 * c2[0], np.float32),
    )
    eph_full = (energy * phi_local).astype(np.float32)
    in_maps = []
    for c in range(NCORES):
        st, ar, ep, su, rel = _host_prep_core(
            c, state, arch, eph_full, surprise, seg_ids)
        f36 = np.concatenate([st.T, ar.T], 0)              # [36, NPAD]
        featsT = f36.reshape(36, NCHUNK, 2, 4096).transpose(2, 0, 1, 3).reshape(
            72, NPAD // 2).astype(ml_dtypes.bfloat16)
        # vtstat [NCHUNK, 128, 64, 15]: lanes 0=1, 1=0(ew), 2:6=0(ew*a),
        # 6:10=a, 10:14=a*a, 14=sur
        acm = ar.reshape(NCHUNK, 2, 32, 128, 4).transpose(0, 3, 2, 1, 4).reshape(
            NCHUNK, 128, 64, 4)
        vst = np.zeros((NCHUNK, 128, 64, 15), np.float32)
        vst[..., 0] = 1.0
        vst[..., 6:10] = acm
        vst[..., 10:14] = acm * acm
        vst[..., 14] = _swz1(su)
        relz = _swz1(rel)  # [NCHUNK, 128, 64]
        ohv = (relz[..., None] == np.arange(W, dtype=np.float32)).astype(
            ml_dtypes.float8_e4m3)
        epz = _swz1(ep)
        cvv = np.concatenate([epz, 0.5 * epz], axis=2)
        in_maps.append(dict(
            featsT=np.ascontiguousarray(featsT),
            vtstat=np.ascontiguousarray(
                vst.reshape(NCHUNK, 128, 960).astype(ml_dtypes.bfloat16)),
            cellvec=np.ascontiguousarray(cvv.astype(ml_dtypes.bfloat16)),
            ohdram=np.ascontiguousarray(ohv.reshape(NCHUNK, 128, 2048)),
            **consts))
    nc = _get_program()
    res = run_bass_kernel_spmd(nc, in_maps, list(range(NCORES)))
    outs = res.results
    couts = [np.asarray(outs[c]["out_cluster"]) for c in range(NCORES)]
    orgs = [np.asarray(outs[c]["out_org"]).reshape(12) for c in range(NCORES)]
    cluster_full = np.concatenate(couts, 0).astype(np.float32)
    p = np.sum(np.stack(orgs, 0), 0, dtype=np.float64)
    Z, G, sphi, scoh, nval, pres = p[0], p[1:5], p[5], p[6], p[7], p[8:12]
    ga = (G / Z).astype(np.float32)
    e = np.exp(ga - ga.max())
    global_arch = (e / e.sum()).astype(np.float32)
    n_valid = max(nval, 1.0)
    avg_phi = sphi / n_valid
    unique = float((pres > 0).sum())
    phi_global = min(1.0, avg_phi * (0.5 + 0.5 * unique / 4.0))
    vert = scoh / n_valid
    self_model = np.array([*global_arch, phi_global, vert], np.float32)
    return np.concatenate([cluster_full.reshape(-1), self_model]).astype(np.float32)
